# revision 2
# baseline (speedup 1.0000x reference)
"""HGNN (2x HypergraphConv) as an 8-core SPMD Bass/Tile kernel, v3.

Layout: one big AllGather of e1 (2 halves, overlapped with A1/B1), W2
applied before the layer-2 exchange, layer-2 phases as narrow (32B)
dma_scatter_add pushes + tiny ReduceScatters.

Phases (per core):
  A1: gather xw1 rows (replicated input, 4 int16 blocks) -> one-hot
      matmul scatter into per-edge-tile PSUM accs -> e1_loc (Binv folded)
  AG: AllGather e1 (2 halves)
  B1: gather e1 rows -> per-node-tile accs -> Dinv,b1,relu -> ^T @ W2
      -> hw2_sb (SBUF-resident)
  A2: push: replicate hw2 rows per entry via one-hot matmul, narrow
      scatter-add into e2w partial [ET,64] -> ReduceScatter -> Binv
  B2: same push from e2w_sb into out partial [NT,64] -> ReduceScatter
      -> Dinv, +b2 -> outT
"""
import sys
import numpy as np
import ml_dtypes
from contextlib import ExitStack

sys.path.insert(0, "/opt/trn_rl_repo")

import concourse.bass as bass  # noqa: E402
import concourse.tile as tile  # noqa: E402
from concourse import bacc, mybir  # noqa: E402

F = 128
N_CLS = 8
BF16 = mybir.dt.bfloat16
F32 = mybir.dt.float32
I16 = mybir.dt.int16

G = 8          # chunks per batched one-hot / compare op
ZG = 16        # chunks per z psum group
CALL_CH = 96   # max chunks per scatter call
BLK = 25088    # int16-indexable dest/table block rows


def cdiv(a, b):
    return -(-a // b)


class CFG:
    def __init__(self, N=100000, E=50000, NC=8):
        self.N, self.E, self.NC = N, E, NC
        self.SN, self.SE = N // NC, E // NC
        self.TN, self.TE = cdiv(self.SN, 128), cdiv(self.SE, 128)
        self.SNP, self.SEP = self.TN * 128, self.TE * 128
        self.NT, self.ET = NC * self.SNP, NC * self.SEP
        # A1 supers (<=7 tiles for PSUM budget) aligned to the AG half
        # boundary (tile 25 of TE=49): halves end after supers 3 and 7.
        self.sup_A = [(0, 7), (7, 7), (14, 7), (21, 4),
                      (25, 7), (32, 7), (39, 7), (46, 3)]
        assert sum(n for _, n in self.sup_A) == self.TE
        self.sup_B = [(i * 7, min(7, self.TN - i * 7))
                      for i in range(cdiv(self.TN, 7))]
        self.H0 = 25 * 128          # e1 AG half0 rows per core (3200)
        self.H1 = self.SEP - self.H0  # 3072
        self.nblk_x = cdiv(self.NT, BLK)   # xw1 blocks (4)
        self.nblk_e = cdiv(self.ET, BLK)   # e2w dest blocks (2)
        self.nblk_n = cdiv(self.NT, BLK)   # out dest blocks (4)


def _balance_perm(deg, NC, S):
    """Sort ids by degree desc, snake-deal across NC shards."""
    n = len(deg)
    order = np.argsort(-deg, kind="stable")
    i = np.arange(n)
    blk, pos = i // NC, i % NC
    corearr = np.where(blk % 2 == 1, NC - 1 - pos, pos)
    perm = np.empty(n, np.int64)
    perm[order] = corearr * S + blk
    return perm


def _wrap16(arr, NC, totch):
    """[NC, totch*128] -> wrapped idx layout [NC, 128, totch*8]."""
    return np.ascontiguousarray(np.tile(
        arr.reshape(NC, totch, 8, 16).transpose(0, 3, 1, 2)
        .reshape(NC, 16, totch * 8), (1, 8, 1)))


def _ranks(g):
    order = np.argsort(g, kind="stable")
    gs = g[order]
    n = len(gs)
    if n == 0:
        return order, np.zeros(0, np.int64)
    starts = np.r_[0, np.flatnonzero(np.diff(gs)) + 1]
    lens = np.diff(np.r_[starts, n])
    rank = np.arange(n) - np.repeat(starts, lens)
    return order, rank


def gather_meta(NC, core, tl, slot, tbl, row, supers, ntbl, TT):
    ns = len(supers)
    sup_of = np.zeros(TT, np.int64)
    for si, (t0, nt) in enumerate(supers):
        sup_of[t0:t0 + nt] = si
    s = sup_of[tl]
    g = ((core * ns + s) * ntbl + tbl) * TT + tl
    cnt = np.bincount(g, minlength=NC * ns * ntbl * TT) \
        .reshape(NC, ns, ntbl, TT)
    nch = cdiv(cnt.max(axis=0), 128)  # [ns, ntbl, TT]

    cb = np.zeros((ns, ntbl, TT), np.int64)
    tile_nch = np.zeros(TT, np.int64)
    supers_struct = []
    totch = 0
    for si, (t0, nt) in enumerate(supers):
        sup = {"tiles": list(range(t0, t0 + nt)), "calls": []}
        for b in range(ntbl):
            ch0 = totch
            tlist = []
            for t in range(t0, t0 + nt):
                n = int(nch[si, b, t])
                if n:
                    cb[si, b, t] = totch
                    tlist.append((t, n, totch - ch0, totch))
                    tile_nch[t] += n
                    totch += n
            sup["calls"].append(
                {"tbl": b, "ch0": ch0, "nch": totch - ch0, "tiles": tlist})
        supers_struct.append(sup)

    order, rank = _ranks(g)
    cbe = cb[s, tbl, tl]
    gpos = (cbe[order] + rank // 128) * 128 + rank % 128
    idx_arr = np.zeros((NC, totch * 128), np.int16)
    idx_arr[core[order], gpos] = row[order].astype(np.int16)
    st_arr = np.full((NC, totch * 128), -1.0, np.float32)
    st_arr[core[order], gpos] = slot[order]
    idxw = _wrap16(idx_arr, NC, totch)
    stw = np.ascontiguousarray(
        st_arr.reshape(NC, totch, 128).transpose(0, 2, 1)
        .astype(ml_dtypes.bfloat16))
    return {"totch": totch, "idx": idxw, "st": stw,
            "supers": supers_struct, "tile_nch": tile_nch}


def scatter_meta(NC, core, stile, srow, db, drow, nblk, TS):
    nsg = cdiv(TS, 8)
    sg = stile // 8
    g = ((core * nsg + sg) * nblk + db) * TS + stile
    cnt = np.bincount(g, minlength=NC * nsg * nblk * TS) \
        .reshape(NC, nsg, nblk, TS)
    nch = cdiv(cnt.max(axis=0), 128)  # [nsg, nblk, TS]

    cb = np.zeros((nblk, TS), np.int64)
    calls = []
    totch = 0
    for s in range(nsg):
        for b in range(nblk):
            ch0 = totch
            chunk_tiles = []
            for t in range(s * 8, min((s + 1) * 8, TS)):
                n = int(nch[s, b, t])
                if n:
                    cb[b, t] = totch
                    chunk_tiles += [t] * n
                    totch += n
            nall = totch - ch0
            for cs in range(0, nall, 96):
                nc_ = min(96, nall - cs)
                calls.append({"db": b, "sg": s, "ch0": ch0 + cs, "nch": nc_,
                              "chunk_tiles": chunk_tiles[cs:cs + nc_]})

    order, rank = _ranks(g)
    cbe = cb[db, stile]
    gpos = (cbe[order] + rank // 128) * 128 + rank % 128
    v_arr = np.full((NC, totch * 128), -1.0, np.float32)
    v_arr[core[order], gpos] = srow[order]
    sidx_arr = np.zeros((NC, totch * 128), np.int16)
    sidx_arr[core[order], gpos] = drow[order].astype(np.int16)
    vw = np.ascontiguousarray(
        v_arr.astype(ml_dtypes.bfloat16).reshape(NC, 1, totch * 128))
    sidxw = _wrap16(sidx_arr, NC, totch)
    return {"totch": totch, "v": vw, "sidx": sidxw, "calls": calls}


def prep(cfg, node_idx, edge_idx):
    N, E, NC = cfg.N, cfg.E, cfg.NC
    node_idx = np.asarray(node_idx, np.int64)
    edge_idx = np.asarray(edge_idx, np.int64)
    D = np.bincount(node_idx, minlength=N).astype(np.float32)
    B = np.bincount(edge_idx, minlength=E).astype(np.float32)
    Dinv = np.where(D > 0, 1.0 / np.maximum(D, 1.0), 0.0).astype(np.float32)
    Binv = np.where(B > 0, 1.0 / np.maximum(B, 1.0), 0.0).astype(np.float32)

    perm_v = _balance_perm(D, NC, cfg.SN)
    perm_e = _balance_perm(B, NC, cfg.SE)
    pv = perm_v[node_idx]
    pe = perm_e[edge_idx]
    vco, nl = pv // cfg.SN, pv % cfg.SN
    eco, el = pe // cfg.SE, pe % cfg.SE
    vgp = vco * cfg.SNP + nl   # padded global node row
    egp = eco * cfg.SEP + el   # padded global edge row

    # A1: gather xw1, sharded by edge owner, dest = local edge tiles
    mA1 = gather_meta(NC, eco, el // 128, (el % 128).astype(np.float32),
                      vgp // BLK, vgp % BLK, cfg.sup_A, cfg.nblk_x, cfg.TE)
    # B1: gather e1 halves, sharded by node owner, dest = local node tiles
    h = (el >= cfg.H0).astype(np.int64)
    hrow = np.where(h == 0, eco * cfg.H0 + el, eco * cfg.H1 + (el - cfg.H0))
    mB1 = gather_meta(NC, vco, nl // 128, (nl % 128).astype(np.float32),
                      h, hrow, cfg.sup_B, 2, cfg.TN)
    # A2: push hw2 -> e2w partial, sharded by node owner
    mA2 = scatter_meta(NC, vco, nl // 128, (nl % 128).astype(np.float32),
                       egp // BLK, egp % BLK, cfg.nblk_e, cfg.TN)
    # B2: push e2w -> out partial, sharded by edge owner
    mB2 = scatter_meta(NC, eco, el // 128, (el % 128).astype(np.float32),
                       vgp // BLK, vgp % BLK, cfg.nblk_n, cfg.TE)

    inv_v = np.empty(N, np.int64)
    inv_v[perm_v] = np.arange(N)
    inv_e = np.empty(E, np.int64)
    inv_e[perm_e] = np.arange(E)

    # per-core scale tables
    binv_p = np.zeros((NC, cfg.SEP), np.float32)
    binv_p[:, :cfg.SE] = Binv[inv_e].reshape(NC, cfg.SE)
    binvA = np.ascontiguousarray(
        binv_p.reshape(NC, cfg.TE, 128).transpose(0, 2, 1))
    dinv_p = np.zeros((NC, cfg.SNP), np.float32)
    dinv_p[:, :cfg.SN] = Dinv[inv_v].reshape(NC, cfg.SN)
    dinvB = np.ascontiguousarray(
        dinv_p.reshape(NC, cfg.TN, 128).transpose(0, 2, 1))

    return {"mA1": mA1, "mB1": mB1, "mA2": mA2, "mB2": mB2,
            "perm_v": perm_v, "inv_v": inv_v, "inv_e": inv_e,
            "binvA": binvA, "dinvB": dinvB}
